# revision 1
# baseline (speedup 1.0000x reference)
"""Trainium2 Bass kernel for nn_AttentionBlock (B=4, C=256, H=W=64, IC=128).

Sharding: 8 cores = 4 batches x 2 row-halves of the N=4096 attention dim.
Each core computes its 2048 rows of the attention output, the final 1x1 conv
(wy), and partial BatchNorm statistics; a tiny AllReduce combines the BN
stats; each core then applies BN + residual and writes its output slice.

Algebraic simplifications vs the reference (all exact):
  - g_b and w_b only add a per-channel constant to wy, which BatchNorm's
    mean subtraction cancels -> dropped.
  - dy_b (phi bias) only adds row-constant terms to the attention logits,
    which softmax cancels -> dropped. Only dx_b (theta bias) is applied.
  - softmax is computed without max-subtraction: logits are bounded
    (|f| < ~70 for randn inputs), well within f32/bf16 exp range.

Layout: channels-on-partitions everywhere. Attention scores are computed
TRANSPOSED (fT[m, n] tiles, m on partitions) so exp(fT) chunks feed the
second matmul (y2 = P @ g) directly as stationary-K operands, transpose-
free. The softmax denominator d[n] = sum_m exp(fT) rides the TensorEngine
with a ones-column stationary; 1/d is broadcast across partitions with a
ones-row matmul.
"""

import sys
import numpy as np

if "/opt/trn_rl_repo" not in sys.path:
    sys.path.insert(0, "/opt/trn_rl_repo")

import concourse.bass as bass
import concourse.bacc as bacc
import concourse.mybir as mybir
import concourse.tile as tile
from concourse.bass_utils import run_bass_kernel_spmd

N_CORES = 8
B, C, HW = 4, 256, 64
N = HW * HW          # 4096 spatial positions per batch
IC = 128             # inter channels
NL = N // 2          # 2048 rows per core
NH = NL // 2         # 1024 cols per attention n-half
EPS = 1e-5
CNT = float(B * N)   # BatchNorm count per channel

f32 = mybir.dt.float32
bf16 = mybir.dt.bfloat16
f16 = mybir.dt.float16
ALU = mybir.AluOpType
ACTF = mybir.ActivationFunctionType


def _mm(nc, out, lhsT, rhs, start=True, stop=True, skip_ldw=False):
    # skip_ldw (LDWEIGHTS elision for same-stationary pairs) is disabled:
    # the Tile scheduler may interleave other PE instructions between the
    # pair, which would corrupt the stationary operand. LDWEIGHTS largely
    # overlaps MATMUL via the PE reorder window anyway.
    return nc.tensor.matmul(out, lhsT, rhs, start=start, stop=stop)


def _build():
    nc = bacc.Bacc("TRN2", target_bir_lowering=False, debug=False,
                   num_devices=N_CORES)

    xl_d = nc.dram_tensor("xl", [C, NL], f32, kind="ExternalInput").ap()
    yl_d = nc.dram_tensor("yl", [C, N], f32, kind="ExternalInput").ap()
    dxwT_d = nc.dram_tensor("dxwT", [C, IC], f32, kind="ExternalInput").ap()
    dywT_d = nc.dram_tensor("dywT", [C, IC], f32, kind="ExternalInput").ap()
    gwT_d = nc.dram_tensor("gwT", [C, IC], f32, kind="ExternalInput").ap()
    wwT_d = nc.dram_tensor("wwT", [IC, C], f32, kind="ExternalInput").ap()
    dxb_d = nc.dram_tensor("dxb", [IC, 1], f32, kind="ExternalInput").ap()
    gamma_d = nc.dram_tensor("gamma", [C, 1], f32, kind="ExternalInput").ap()
    beta_d = nc.dram_tensor("beta", [C, 1], f32, kind="ExternalInput").ap()
    out_d = nc.dram_tensor("out", [C, NL], f32, kind="ExternalOutput").ap()

    with tile.TileContext(nc) as tc:
        _emit(nc, tc, xl_d, yl_d, dxwT_d, dywT_d, gwT_d, wwT_d, dxb_d,
              gamma_d, beta_d, out_d)
    nc.compile()
    return nc


def _emit(nc, tc, xl_d, yl_d, dxwT_d, dywT_d, gwT_d, wwT_d, dxb_d,
          gamma_d, beta_d, out_d):
    with (
        tc.tile_pool(name="sb_w", bufs=1) as wp,        # weights + tiny tiles
        tc.tile_pool(name="sb_x", bufs=2) as xp,        # x / y staging
        tc.tile_pool(name="sb_a", bufs=1) as ap_,       # theta/phi/g activations
        tc.tile_pool(name="sb_e", bufs=3) as ep,        # exp tiles
        tc.tile_pool(name="sb_m", bufs=2) as mp,        # misc per-half tiles
        tc.tile_pool(name="sb_bn", bufs=1) as bp,       # bn tiny tiles
        tc.tile_pool(name="ps", bufs=2, space="PSUM") as pp,
        tc.tile_pool(name="dram", bufs=1, space="DRAM") as dr,
    ):
        # ---------------- load + cast inputs ----------------
        xl_t, xh_t, yh_t = [], [], []
        for i in range(2):
            xt = xp.tile([128, NL], f32, tag="xl")
            nc.sync.dma_start(xt[:], xl_d[128 * i:128 * (i + 1), :])
            xl_t.append(xt)
            xh = xp.tile([128, NL], f16, tag="xh")
            nc.vector.tensor_copy(xh[:], xt[:])
            xh_t.append(xh)
        for i in range(2):
            yt = xp.tile([128, N], f32, tag="yl")
            nc.sync.dma_start(yt[:], yl_d[128 * i:128 * (i + 1), :])
            yh = xp.tile([128, N], f16, tag="yh")
            nc.vector.tensor_copy(yh[:], yt[:])
            yh_t.append(yh)

        wdx_h, wdy_h, wg_h = [], [], []
        for i in range(2):
            for nm, (dst, src_d) in (("dx", (wdx_h, dxwT_d)),
                                     ("dy", (wdy_h, dywT_d)),
                                     ("g", (wg_h, gwT_d))):
                wt = wp.tile([128, IC], f32, tag="wtmp", bufs=2)
                nc.sync.dma_start(wt[:], src_d[128 * i:128 * (i + 1), :])
                wh = wp.tile([128, IC], f16, tag=f"wh_{nm}{i}")
                nc.vector.tensor_copy(wh[:], wt[:])
                dst.append(wh)
        wwT_f = wp.tile([IC, C], f32, tag="wwT_f")
        nc.sync.dma_start(wwT_f[:], wwT_d[:])
        wwT_b = wp.tile([IC, C], bf16, tag="wwT_b")
        nc.vector.tensor_copy(wwT_b[:], wwT_f[:])

        dxb_t = wp.tile([IC, 1], f32, tag="dxb")
        nc.sync.dma_start(dxb_t[:], dxb_d[:])
        gamma_t, beta_t = [], []
        for i in range(2):
            gt = wp.tile([128, 1], f32, tag=f"gam{i}")
            nc.sync.dma_start(gt[:], gamma_d[128 * i:128 * (i + 1), :])
            gamma_t.append(gt)
            bt = wp.tile([128, 1], f32, tag=f"bet{i}")
            nc.sync.dma_start(bt[:], beta_d[128 * i:128 * (i + 1), :])
            beta_t.append(bt)

        ones_m = wp.tile([128, 1], bf16, tag="ones_m")   # d-matmul stationary
        nc.vector.memset(ones_m[:], 1.0)
        ones_r = wp.tile([1, 128], f32, tag="ones_r")    # rinv bcast stationary
        nc.vector.memset(ones_r[:], 1.0)

        # ---------------- projections ----------------
        # PSUM tags (one unified pool): "half" [128,1024] bufs=2 (4 banks),
        # "quar" [128,512] bufs=2 (2 banks), "dv" [1/128,512] bufs=2 (2 banks)
        theta_h = ap_.tile([IC, NL], f16, tag="theta")
        phi_h = ap_.tile([IC, N], f16, tag="phi")
        g_sb = ap_.tile([128, N], bf16, tag="g")   # 32 chunks [m128, ic128]

        # g projection: chunk m -> g[m128, ic] = sum_c y[c, m128].T @ gwT[c, ic]
        for t in range(8):                   # 8 psum tiles, 4 m-chunks each
            gp = pp.tile([128, 512], f32, tag="quar", name=f"gp{t}")
            for j in range(4):
                m = 4 * t + j
                for i in range(2):
                    _mm(nc, gp[:, 128 * j:128 * (j + 1)],
                        yh_t[i][:, 128 * m:128 * (m + 1)], wg_h[i][:],
                        start=(i == 0), stop=(i == 1))
            nc.scalar.copy(g_sb[:, 512 * t:512 * (t + 1)], gp[:])

        # theta: [ic, nl] = dxwT.T @ x  (+ dx_b), 2 psum tiles of 1024
        for t in range(2):
            tp = pp.tile([128, 1024], f32, tag="half", name=f"thp{t}")
            for i in range(2):
                for j in range(2):
                    _mm(nc, tp[:, 512 * j:512 * (j + 1)], wdx_h[i][:],
                        xh_t[i][:, 1024 * t + 512 * j:1024 * t + 512 * (j + 1)],
                        start=(i == 0), stop=(i == 1), skip_ldw=(j == 1))
            nc.vector.tensor_scalar(theta_h[:, 1024 * t:1024 * (t + 1)], tp[:],
                                    dxb_t[:], None, ALU.add)

        # phi: [ic, n] = dywT.T @ y  (bias dropped: softmax-invariant)
        for t in range(4):
            php = pp.tile([128, 1024], f32, tag="half", name=f"php{t}")
            for i in range(2):
                for j in range(2):
                    _mm(nc, php[:, 512 * j:512 * (j + 1)], wdy_h[i][:],
                        yh_t[i][:, 1024 * t + 512 * j:1024 * t + 512 * (j + 1)],
                        start=(i == 0), stop=(i == 1), skip_ldw=(j == 1))
            nc.vector.tensor_copy(phi_h[:, 1024 * t:1024 * (t + 1)], php[:])

        # ---------------- attention ----------------
        wy_sb = [mp.tile([128, NL], f32, tag=f"wy{c}", bufs=1, name=f"wy_sb{c}")
                 for c in range(2)]
        packed = bp.tile([128, 16], f32, tag="packed")  # accum_out columns

        for h2 in range(2):
            n0 = NH * h2
            y2_ps = [pp.tile([IC, 512], f32, tag="quar", name=f"y2p{h2}_{j}")
                     for j in range(2)]
            d_ps = [pp.tile([1, 512], f32, tag="dv", name=f"dp{h2}_{j}")
                    for j in range(2)]
            for m in range(32):
                ft = pp.tile([128, 1024], f32, tag="half", name=f"ft{h2}_{m}")
                for j in range(2):
                    _mm(nc, ft[:, 512 * j:512 * (j + 1)],
                        phi_h[:, 128 * m:128 * (m + 1)],
                        theta_h[:, n0 + 512 * j:n0 + 512 * (j + 1)],
                        skip_ldw=(j == 1))
                expP = ep.tile([128, 1024], bf16, tag="exp", name=f"ex{h2}_{m}")
                nc.scalar.activation(expP[:], ft[:], ACTF.Exp)
                for j in range(2):
                    _mm(nc, y2_ps[j][:], g_sb[:, 128 * m:128 * (m + 1)],
                        expP[:, 512 * j:512 * (j + 1)],
                        start=(m == 0), stop=(m == 31), skip_ldw=(j == 1))
                for j in range(2):
                    _mm(nc, d_ps[j][:], ones_m[:],
                        expP[:, 512 * j:512 * (j + 1)],
                        start=(m == 0), stop=(m == 31), skip_ldw=(j == 1))

            # normalize y2 /= d (per 512-piece), then wy + stats
            y2sb = mp.tile([IC, NH], bf16, tag="y2sb", name=f"y2sb{h2}")
            for j in range(2):
                rinv = mp.tile([1, 512], f32, tag="rinv", name=f"ri{h2}_{j}")
                nc.vector.reciprocal_approx_fast(rinv[:], d_ps[j][:])
                rb_ps = pp.tile([128, 512], f32, tag="dv", name=f"rb{h2}_{j}")
                _mm(nc, rb_ps[:], ones_r[:], rinv[:], skip_ldw=(j == 1))
                rb_sb = mp.tile([128, 512], f32, tag="rb", name=f"rbs{h2}_{j}")
                nc.vector.tensor_copy(rb_sb[:], rb_ps[:])
                nc.vector.tensor_tensor(y2sb[:, 512 * j:512 * (j + 1)],
                                        y2_ps[j][:], rb_sb[:], op=ALU.mult)

            for c in range(2):
                wyp = pp.tile([128, NH], f32, tag="half", name=f"wyp{h2}_{c}")
                for j in range(2):
                    _mm(nc, wyp[:, 512 * j:512 * (j + 1)],
                        wwT_b[:, 128 * c:128 * (c + 1)],
                        y2sb[:, 512 * j:512 * (j + 1)], skip_ldw=(j == 1))
                k = 4 * c + 2 * h2
                nc.scalar.activation(wy_sb[c][:, n0:n0 + NH], wyp[:],
                                     ACTF.Copy, accum_out=packed[:, k:k + 1])
                sq = ep.tile([128, NH], bf16, tag="sqscratch", bufs=2,
                             name=f"sq{h2}_{c}")
                nc.scalar.activation(sq[:], wyp[:], ACTF.Square,
                                     accum_out=packed[:, k + 1:k + 2])

            # per-half stats AllReduce: half 0 overlaps half 1's compute
            arp = bp.tile([128, 4], f32, tag=f"arp{h2}", name=f"arp{h2}")
            for c in range(2):
                k = 4 * c + 2 * h2
                nc.vector.tensor_copy(arp[:, 2 * c:2 * c + 2],
                                      packed[:, k:k + 2])
            ar_in = dr.tile([128, 4], f32, name=f"ar_in{h2}")
            ar_out = dr.tile([128, 4], f32, name=f"ar_out{h2}")
            nc.gpsimd.dma_start(ar_in[:], arp[:])
            nc.gpsimd.collective_compute(
                "AllReduce", ALU.add,
                replica_groups=[list(range(N_CORES))],
                ins=[ar_in.opt()], outs=[ar_out.opt()])
            gsb = bp.tile([128, 4], f32, tag=f"gsb{h2}", name=f"gsb{h2}")
            nc.gpsimd.dma_start(gsb[:], ar_out[:])
            if h2 == 0:
                stats_a = gsb
            else:
                stats_b = gsb

        stats_g = bp.tile([128, 4], f32, tag="stats_g")
        nc.vector.tensor_tensor(stats_g[:], stats_a[:], stats_b[:], op=ALU.add)

        # ---------------- BN apply + residual ----------------
        for c in range(2):
            mean = bp.tile([128, 1], f32, tag=f"mean{c}")
            nc.vector.tensor_scalar(mean[:], stats_g[:, 2 * c:2 * c + 1],
                                    1.0 / CNT, None, ALU.mult)
            msq = bp.tile([128, 1], f32, tag=f"msq{c}")
            nc.vector.tensor_scalar(msq[:], stats_g[:, 2 * c + 1:2 * c + 2],
                                    1.0 / CNT, None, ALU.mult)
            m2 = bp.tile([128, 1], f32, tag=f"m2{c}")
            nc.vector.tensor_tensor(m2[:], mean[:], mean[:], op=ALU.mult)
            var = bp.tile([128, 1], f32, tag=f"var{c}")
            nc.vector.tensor_tensor(var[:], msq[:], m2[:], op=ALU.subtract)
            varep = bp.tile([128, 1], f32, tag=f"varep{c}")
            nc.vector.tensor_scalar(varep[:], var[:], float(EPS), None, ALU.add)
            sd = bp.tile([128, 1], f32, tag=f"sd{c}")
            nc.scalar.activation(sd[:], varep[:], ACTF.Sqrt)
            rstd = bp.tile([128, 1], f32, tag=f"rstd{c}")
            nc.vector.reciprocal(rstd[:], sd[:])
            scale = bp.tile([128, 1], f32, tag=f"scale{c}")
            nc.vector.tensor_tensor(scale[:], gamma_t[c][:], rstd[:], op=ALU.mult)
            msc = bp.tile([128, 1], f32, tag=f"msc{c}")
            nc.vector.tensor_tensor(msc[:], mean[:], scale[:], op=ALU.mult)
            shift = bp.tile([128, 1], f32, tag=f"shift{c}")
            nc.vector.tensor_tensor(shift[:], beta_t[c][:], msc[:], op=ALU.subtract)

            out_t = mp.tile([128, NL], f32, tag=f"out{c}", bufs=1, name=f"out{c}")
            for k in range(2):
                sl = slice(1024 * k, 1024 * (k + 1))
                nc.vector.affine_then_add(out_t[:, sl], wy_sb[c][:, sl],
                                          xl_t[c][:, sl], scale[:], shift[:])
                nc.sync.dma_start(out_d[128 * c:128 * (c + 1), sl],
                                  out_t[:, sl])


_NC_CACHE = None


def _get_nc():
    global _NC_CACHE
    if _NC_CACHE is None:
        _NC_CACHE = _build()
    return _NC_CACHE


def shard_inputs(inputs):
    x = np.ascontiguousarray(inputs["x"], dtype=np.float32).reshape(B, C, N)
    y = np.ascontiguousarray(inputs["y"], dtype=np.float32).reshape(B, C, N)
    dxwT = np.ascontiguousarray(np.asarray(inputs["dx_w"]).T.astype(np.float32))
    dywT = np.ascontiguousarray(np.asarray(inputs["dy_w"]).T.astype(np.float32))
    gwT = np.ascontiguousarray(np.asarray(inputs["g_w"]).T.astype(np.float32))
    wwT = np.ascontiguousarray(np.asarray(inputs["w_w"]).T.astype(np.float32))
    dxb = np.ascontiguousarray(inputs["dx_b"], dtype=np.float32).reshape(IC, 1)
    gamma = np.ascontiguousarray(inputs["bn_gamma"], dtype=np.float32).reshape(C, 1)
    beta = np.ascontiguousarray(inputs["bn_beta"], dtype=np.float32).reshape(C, 1)

    in_maps = []
    for core in range(N_CORES):
        b, h = divmod(core, 2)
        in_maps.append({
            "xl": np.ascontiguousarray(x[b][:, h * NL:(h + 1) * NL]),
            "yl": y[b],
            "dxwT": dxwT, "dywT": dywT, "gwT": gwT, "wwT": wwT,
            "dxb": dxb, "gamma": gamma, "beta": beta,
        })
    return in_maps


def run(inputs, **kw):
    """Run on hardware; returns (full_output, BassKernelResults)."""
    nc = _get_nc()
    in_maps = shard_inputs(inputs)
    r = run_bass_kernel_spmd(nc, in_maps, core_ids=list(range(N_CORES)), **kw)
    out = np.empty((B, C, N), np.float32)
    for core in range(N_CORES):
        b, h = divmod(core, 2)
        out[b][:, h * NL:(h + 1) * NL] = r.results[core]["out"]
    return out.reshape(B, C, HW, HW), r


def kernel(**inputs):
    out, _ = run(inputs)
    return out

